# revision 11
# baseline (speedup 1.0000x reference)
"""GIN 2-layer encoder (gnn_message_passing) on 8 TRN2 NeuronCores, Bass/Tile SPMD.

Strategy (sharding_hint: partition nodes + incident edges, replicate params,
all-reduce BN stats, exchange activations):
- Nodes sharded 12500/core, padded to 12544 = 98*128. Edges assigned to the
  core owning dst.
- GIN linearity: mlp(h + agg(h)) with first op Linear => y = h@W1 per node,
  then z = y + segsum(y[src]) + b1. So per layer: local y matmul -> AllGather
  y table -> gather y[src] rows (dma_gather, 256B rows, 4 int16 subtable
  passes) -> segment-sum into per-dst-tile PSUM via is_equal masks (bf16) +
  PE matmuls (no scatter; dst grouped by 128-node tile on host).
- BN (training stats over all N): per-feature sum/sumsq via ones-matmul,
  AllReduce, fold scale/shift applied in transposed space before the W2 matmul.
- Global add pool: mask matmuls vs batch ids into 8 global graph windows
  (psum [128, 8*64]) + AllReduce.
SPMD: one program for all cores; per-(subtable, dst-tile) chunk counts are
padded to the max across cores so the structure is identical; per-core data
(indices, mask columns) differs only in tensor contents.
Host preprocessing is index-only (grouping/sorting/layout of edge indices).
"""
import sys
sys.path.insert(0, "/opt/trn_rl_repo")

import numpy as np

# ---------------- dimensions (hardcoded per contract) ----------------
N = 100000
E = 3200000
G = 1024
D_IN = 128
H = 64
BN_EPS = 1e-5
NCORES = 8

P = 128
KG = 4096              # idxs per dma_gather call
SUBC = KG // P         # sub-chunks per gather (32)
NSUB = 4               # gather subtables (int16 idx limit)


def _derived():
    global NSH, NS, NT, NFULL, ST, NW
    NSH = N // NCORES                      # real nodes per core
    NS = ((NSH + P - 1) // P) * P          # padded nodes per core
    NT = NS // P                           # node tiles per core
    NFULL = NS * NCORES                    # padded total
    assert NFULL % NSUB == 0
    ST = NFULL // NSUB                     # subtable rows
    assert ST <= 32768, ST
    NW = (G + P - 1) // P                  # pooling windows (global)


_derived()


def _wrap_idxs(idx):
    """[K] -> [128, K/16] int16: idx j at partition j%16, col j//16, x8 repl."""
    k = idx.shape[0]
    w = idx.astype(np.int16).reshape(k // 16, 16).T
    return np.tile(w, (8, 1))


def preprocess(edge_index, batch):
    """Index-only host prep -> (structure, per-core arrays)."""
    src = np.asarray(edge_index[0], np.int64)
    dst = np.asarray(edge_index[1], np.int64)
    batch = np.asarray(batch, np.int64)
    core = dst // NSH
    dl = dst - core * NSH
    srcp = (src // NSH) * NS + (src % NSH)
    s_tab = srcp // ST
    s_loc = srcp - s_tab * ST
    t_dst = dl // P
    w_dst = dl - t_dst * P

    # per-core, per-(s,t) edge lists
    counts = np.zeros((NCORES, NSUB, NT), np.int64)
    per = {}
    for c in range(NCORES):
        m = core == c
        key = (s_tab[m] * NT + t_dst[m]).astype(np.int64)
        order = np.argsort(key, kind="stable")
        sl_, td_, wd_, key_ = s_loc[m][order], t_dst[m][order], w_dst[m][order], key[order]
        ug, starts, cnts = np.unique(key_, return_index=True, return_counts=True)
        counts[c][ug // NT, ug % NT] = cnts
        per[c] = (sl_, wd_, {int(g): (int(a), int(n))
                             for g, a, n in zip(ug, starts, cnts)})

    # uniform chunk counts: max over cores, >= 1
    nch = np.maximum(1, (counts.max(axis=0) + P - 1) // P)  # [NSUB, NT]
    runs = [[(t, int(nch[s, t])) for t in range(NT)] for s in range(NSUB)]
    ng = []
    for s in range(NSUB):
        tot = int(nch[s].sum()) * P
        ng.append((tot + KG - 1) // KG)

    cores = []
    for c in range(NCORES):
        sl_, wd_, groups = per[c]
        gidx_parts, dl_parts = [], []
        for s in range(NSUB):
            ilen = 0
            for t in range(NT):
                g = s * NT + t
                a, n = groups.get(g, (0, 0))
                want = int(nch[s, t]) * P
                ii = np.zeros(want, np.int64)
                dd = np.full(want, 999, np.int64)
                ii[:n] = sl_[a:a + n]
                dd[:n] = wd_[a:a + n]
                gidx_parts.append(ii)
                dl_parts.append(dd)
                ilen += want
            pad = ng[s] * KG - ilen
            if pad:
                gidx_parts.append(np.zeros(pad, np.int64))
                dl_parts.append(np.full(pad, 999, np.int64))
        gidx = np.concatenate(gidx_parts)
        dloc = np.concatenate(dl_parts)
        nchk = dloc.shape[0] // P
        dloc_t = dloc.reshape(nchk, P).T  # [128, nchunks]

        bown = batch[c * NSH:(c + 1) * NSH]
        bpad = np.concatenate([bown, np.full(NS - NSH, 10 ** 6)])
        boff = np.zeros((P, NT * NW), np.int64)
        for t in range(NT):
            seg = bpad[t * P:(t + 1) * P]
            for w in range(NW):
                boff[:, t * NW + w] = seg - w * P
        cores.append(dict(gidx=_wrap_idxs(gidx), dloc=dloc_t, boff=boff))
    return dict(runs=runs, ng=ng, cores=cores,
                idx_cols=cores[0]["gidx"].shape[1],
                chunk_cols=cores[0]["dloc"].shape[1])


def build_nc(struct, stub=False):
    from concourse import mybir
    from concourse import bacc
    import concourse.tile as tile
    import contextlib

    f32 = mybir.dt.float32
    bf16 = mybir.dt.bfloat16
    i16 = mybir.dt.int16
    AF = mybir.ActivationFunctionType

    runs, ng = struct["runs"], struct["ng"]
    idx_cols, chunk_cols = struct["idx_cols"], struct["chunk_cols"]
    max_ng = max(ng) if ng else 1

    nc = bacc.Bacc("TRN2", target_bir_lowering=False, debug=False,
                   num_devices=NCORES)

    xT = nc.dram_tensor("xT", [D_IN, NS], f32, kind="ExternalInput")
    Wd = {}
    for l in range(2):
        kin = D_IN if l == 0 else H
        for nm, shp in (("W1", [kin, H]), ("W2", [H, H]), ("b1", [P, H]),
                        ("b2", [P, H]), ("g", [H, 1]), ("be", [H, 1])):
            Wd[(nm, l)] = nc.dram_tensor(f"{nm}_{l}", shp, f32,
                                         kind="ExternalInput")
    iota_d = nc.dram_tensor("iota", [P, P], bf16, kind="ExternalInput")
    ident_d = nc.dram_tensor("ident", [P, P], f32, kind="ExternalInput")
    ones_d = nc.dram_tensor("ones", [P, 1], f32, kind="ExternalInput")
    pmask_d = nc.dram_tensor("pmask", [P, 1], f32, kind="ExternalInput")
    gidx_d = nc.dram_tensor("gidx", [P, idx_cols], i16, kind="ExternalInput")
    dloc_d = nc.dram_tensor("dloc", [P, chunk_cols], bf16, kind="ExternalInput")
    boff_d = nc.dram_tensor("boff", [P, NT * NW], bf16, kind="ExternalInput")

    out_h = nc.dram_tensor("out_h", [NS, H], f32, kind="ExternalOutput")
    out_xg = nc.dram_tensor("out_xg", [G, H], f32, kind="ExternalOutput")

    ccy_in = [nc.dram_tensor(f"ccy_in{l}", [NS, H], f32) for l in range(2)]
    ccy_out = [nc.dram_tensor(f"ccy_out{l}", [NFULL, H], f32,
                              addr_space="Shared") for l in range(2)]
    ccs_in = [nc.dram_tensor(f"ccs_in{l}", [H, 2], f32) for l in range(2)]
    ccs_out = [nc.dram_tensor(f"ccs_out{l}", [H, 2], f32,
                              addr_space="Shared") for l in range(2)]
    ccp_in = nc.dram_tensor("ccp_in", [G, H], f32)
    ccp_out = nc.dram_tensor("ccp_out", [G, H], f32, addr_space="Shared")

    RG = [list(range(NCORES))]

    if stub:
        with tile.TileContext(nc) as tc:
            with tc.tile_pool(name="z", bufs=1) as zp:
                zt = zp.tile([P, NT * H], mybir.dt.float32)
                nc.vector.memset(zt[:], 0.0)
                nc.sync.dma_start(
                    out=out_h[:].rearrange("(t p) d -> p t d", p=P),
                    in_=zt[:].rearrange("p (t d) -> p t d", d=H))
                zg = zp.tile([P, NW * H], mybir.dt.float32)
                nc.vector.memset(zg[:], 0.0)
                nc.sync.dma_start(
                    out=out_xg[:].rearrange("(w p) d -> p w d", p=P),
                    in_=zg[:].rearrange("p (w d) -> p w d", d=H))
        nc.compile()
        return nc

    with tile.TileContext(nc) as tc:
        with contextlib.ExitStack() as ctx:
            cpool = ctx.enter_context(tc.tile_pool(name="const", bufs=1))
            big = ctx.enter_context(tc.tile_pool(name="big", bufs=1))
            xpool = ctx.enter_context(tc.tile_pool(name="xs", bufs=2))
            ipool = ctx.enter_context(tc.tile_pool(name="idx", bufs=2))
            vpool = ctx.enter_context(tc.tile_pool(name="vals", bufs=4))
            bpool = ctx.enter_context(tc.tile_pool(name="valsbf", bufs=3))
            mpool = ctx.enter_context(tc.tile_pool(name="masks", bufs=3))
            spool = ctx.enter_context(tc.tile_pool(name="small", bufs=3))
            pacc = ctx.enter_context(tc.tile_pool(name="pacc", bufs=3,
                                                  space="PSUM"))
            pmisc = ctx.enter_context(tc.tile_pool(name="pmisc", bufs=1,
                                                   space="PSUM"))
            pstat = ctx.enter_context(tc.tile_pool(name="pstat", bufs=1,
                                                   space="PSUM"))
            ppool = ctx.enter_context(tc.tile_pool(name="ppool", bufs=2,
                                                   space="PSUM"))

            iota = cpool.tile([P, P], bf16)
            nc.sync.dma_start(out=iota[:], in_=iota_d[:])
            ident = cpool.tile([P, P], f32)
            nc.sync.dma_start(out=ident[:], in_=ident_d[:])
            onesc = cpool.tile([P, 1], f32)
            nc.sync.dma_start(out=onesc[:], in_=ones_d[:])
            pmask = cpool.tile([P, 1], f32)
            nc.sync.dma_start(out=pmask[:], in_=pmask_d[:])
            dloc = cpool.tile([P, chunk_cols], bf16)
            nc.sync.dma_start(out=dloc[:], in_=dloc_d[:])
            boffs = cpool.tile([P, NT * NW], bf16)
            nc.sync.dma_start(out=boffs[:], in_=boff_d[:])
            wt = {}
            for l in range(2):
                kin = D_IN if l == 0 else H
                for nm, shp in (("W1", [kin, H]), ("W2", [H, H]),
                                ("b1", [P, H]), ("b2", [P, H]),
                                ("g", [H, 1]), ("be", [H, 1])):
                    t_ = cpool.tile(shp, f32, tag=f"{nm}_{l}")
                    nc.sync.dma_start(out=t_[:], in_=Wd[(nm, l)][:])
                    wt[(nm, l)] = t_

            ybuf = big.tile([P, NT * H], f32, tag="ybuf")
            abuf = big.tile([P, NT * H], f32, tag="abuf")

            XG = 8  # x tiles loaded per DMA

            def y_from_x():
                for t0 in range(0, NT, XG):
                    n = min(XG, NT - t0)
                    xt = xpool.tile([P, XG * P], f32, tag="xT")
                    nc.sync.dma_start(out=xt[:, :n * P],
                                      in_=xT[:, t0 * P:(t0 + n) * P])
                    for k in range(n):
                        t = t0 + k
                        ps = pmisc.tile([P, H], f32, tag="mp")
                        nc.tensor.matmul(ps[:], lhsT=xt[:, k * P:(k + 1) * P],
                                         rhs=wt[("W1", 0)][:],
                                         start=True, stop=True)
                        nc.vector.tensor_copy(out=ybuf[:, t * H:(t + 1) * H],
                                              in_=ps[:])

            def y_from_h(l, hsrc, ydst):
                for t in range(NT):
                    ps = pmisc.tile([H, P], f32, tag="mp")
                    nc.tensor.transpose(out=ps[:],
                                        in_=hsrc[:, t * H:(t + 1) * H],
                                        identity=ident[:])
                    hT = spool.tile([H, P], f32, tag="hT")
                    nc.vector.tensor_copy(out=hT[:], in_=ps[:])
                    ps2 = pmisc.tile([P, H], f32, tag="mp")
                    nc.tensor.matmul(ps2[:], lhsT=hT[:], rhs=wt[("W1", l)][:],
                                     start=True, stop=True)
                    nc.vector.tensor_copy(out=ydst[:, t * H:(t + 1) * H],
                                          in_=ps2[:])

            def allgather_y(l, ysrc):
                nc.sync.dma_start(
                    out=ccy_in[l][:].rearrange("(t p) d -> p t d", p=P),
                    in_=ysrc[:].rearrange("p (t d) -> p t d", d=H))
                nc.gpsimd.collective_compute(
                    "AllGather", nc_alu("bypass"), replica_groups=RG,
                    ins=[ccy_in[l][:]], outs=[ccy_out[l][:]])

            def nc_alu(name):
                return getattr(mybir.AluOpType, name)

            def edge_phase(l, aggdst):
                nc.vector.memset(aggdst[:], 0.0)
                yfull = ccy_out[l]
                icol = 0   # idx col offset in gidx_d (16 idx / col)
                cbase = 0  # chunk col offset in dloc
                for s in range(NSUB):
                    if ng[s] == 0:
                        continue
                    ncols = ng[s] * KG // 16
                    idxt = ipool.tile([P, max_ng * KG // 16], i16, tag="gidx")
                    nc.sync.dma_start(out=idxt[:, :ncols],
                                      in_=gidx_d[:, icol:icol + ncols])
                    chunk_meta = []
                    for t, nchk in runs[s]:
                        for k in range(nchk):
                            chunk_meta.append((t, k == 0, k == nchk - 1))
                    ps_cur = None
                    for gci in range(ng[s]):
                        vals = vpool.tile([P, SUBC * H], f32, tag="vals")
                        nc.gpsimd.dma_gather(
                            vals[:].rearrange("p (c d) -> p c d", d=H),
                            yfull[s * ST:(s + 1) * ST, :],
                            idxt[:, gci * KG // 16:(gci + 1) * KG // 16],
                            KG, KG, H, single_packet=False)
                        vb = bpool.tile([P, SUBC * H], bf16, tag="valsbf")
                        nc.vector.tensor_copy(out=vb[:], in_=vals[:])
                        mk = mpool.tile([P, SUBC * P], bf16, tag="mask")
                        cb = cbase + gci * SUBC
                        nc.vector.tensor_tensor(
                            out=mk[:].rearrange("p (c j) -> p c j", j=P),
                            in0=dloc[:, cb:cb + SUBC].to_broadcast([P, SUBC, P]),
                            in1=iota[:].rearrange("p (c j) -> p c j", c=1)
                                .to_broadcast([P, SUBC, P]),
                            op=nc_alu("is_equal"))
                        for loc in range(SUBC):
                            ci = gci * SUBC + loc
                            if ci >= len(chunk_meta):
                                break
                            t, first, last = chunk_meta[ci]
                            if first:
                                ps_cur = pacc.tile([P, H], f32, tag="acc")
                            nc.tensor.matmul(
                                ps_cur[:], lhsT=mk[:, loc * P:(loc + 1) * P],
                                rhs=vb[:, loc * H:(loc + 1) * H],
                                start=first, stop=last)
                            if last:
                                nc.vector.tensor_add(
                                    out=aggdst[:, t * H:(t + 1) * H],
                                    in0=aggdst[:, t * H:(t + 1) * H],
                                    in1=ps_cur[:])
                    icol += ncols
                    cbase += ng[s] * SUBC

            def dense_phase(l, ysrc, aggbuf, hdst):
                sums0 = pstat.tile([H, 1], f32, tag="stats0")
                sums1 = pstat.tile([H, 1], f32, tag="stats1")
                for t in range(NT):
                    a = aggbuf[:, t * H:(t + 1) * H]
                    nc.vector.tensor_add(out=a, in0=a,
                                         in1=ysrc[:, t * H:(t + 1) * H])
                    nc.vector.tensor_add(out=a, in0=a, in1=wt[("b1", l)][:])
                    nc.scalar.activation(out=a, in_=a, func=AF.Relu)
                    if t == NT - 1:
                        nc.vector.tensor_mul(out=a, in0=a,
                                             in1=pmask[:].to_broadcast([P, H]))
                    sq = spool.tile([P, H], f32, tag="sq")
                    nc.vector.tensor_mul(out=sq[:], in0=a, in1=a)
                    nc.tensor.matmul(sums0[:], lhsT=a, rhs=onesc[:],
                                     start=(t == 0), stop=(t == NT - 1))
                    nc.tensor.matmul(sums1[:], lhsT=sq[:], rhs=onesc[:],
                                     start=(t == 0), stop=(t == NT - 1))
                st_sb = spool.tile([H, 2], f32, tag="stsb")
                nc.vector.tensor_copy(out=st_sb[:, 0:1], in_=sums0[:])
                nc.vector.tensor_copy(out=st_sb[:, 1:2], in_=sums1[:])
                nc.sync.dma_start(out=ccs_in[l][:], in_=st_sb[:])
                nc.gpsimd.collective_compute(
                    "AllReduce", nc_alu("add"), replica_groups=RG,
                    ins=[ccs_in[l][:]], outs=[ccs_out[l][:]])
                st2 = spool.tile([H, 2], f32, tag="st2")
                nc.sync.dma_start(out=st2[:], in_=ccs_out[l][:])
                mean = spool.tile([H, 1], f32, tag="mean")
                nc.vector.tensor_scalar_mul(out=mean[:], in0=st2[:, 0:1],
                                            scalar1=1.0 / N)
                ex2 = spool.tile([H, 1], f32, tag="ex2")
                nc.vector.tensor_scalar_mul(out=ex2[:], in0=st2[:, 1:2],
                                            scalar1=1.0 / N)
                var = spool.tile([H, 1], f32, tag="var")
                nc.vector.tensor_mul(out=var[:], in0=mean[:], in1=mean[:])
                nc.vector.tensor_sub(out=var[:], in0=ex2[:], in1=var[:])
                nc.vector.tensor_scalar_add(out=var[:], in0=var[:],
                                            scalar1=BN_EPS)
                rstd = spool.tile([H, 1], f32, tag="rstd")
                nc.scalar.activation(out=rstd[:], in_=var[:], func=AF.Sqrt)
                nc.vector.reciprocal(out=rstd[:], in_=rstd[:])
                scale = spool.tile([H, 1], f32, tag="scale")
                nc.vector.tensor_mul(out=scale[:], in0=wt[("g", l)][:],
                                     in1=rstd[:])
                shift = spool.tile([H, 1], f32, tag="shift")
                nc.vector.tensor_mul(out=shift[:], in0=mean[:], in1=scale[:])
                nc.vector.tensor_sub(out=shift[:], in0=wt[("be", l)][:],
                                     in1=shift[:])
                for t in range(NT):
                    ps = pmisc.tile([H, P], f32, tag="mp")
                    nc.tensor.transpose(out=ps[:],
                                        in_=aggbuf[:, t * H:(t + 1) * H],
                                        identity=ident[:])
                    aT = spool.tile([H, P], f32, tag="aT")
                    nc.vector.tensor_mul(out=aT[:], in0=ps[:],
                                         in1=scale[:].to_broadcast([H, P]))
                    nc.vector.tensor_add(out=aT[:], in0=aT[:],
                                         in1=shift[:].to_broadcast([H, P]))
                    ps2 = pmisc.tile([P, H], f32, tag="mp")
                    nc.tensor.matmul(ps2[:], lhsT=aT[:], rhs=wt[("W2", l)][:],
                                     start=True, stop=True)
                    hsl = hdst[:, t * H:(t + 1) * H]
                    nc.vector.tensor_add(out=hsl, in0=ps2[:],
                                         in1=wt[("b2", l)][:])
                    nc.scalar.activation(out=hsl, in_=hsl, func=AF.Relu)

            # ---------------- layers ----------------
            y_from_x()
            allgather_y(0, ybuf)
            edge_phase(0, abuf)
            dense_phase(0, ybuf, abuf, ybuf)     # h0 -> ybuf
            y_from_h(1, ybuf, abuf)              # y2 -> abuf
            allgather_y(1, abuf)
            edge_phase(1, ybuf)                  # agg2 -> ybuf (h0 dead)
            dense_phase(1, abuf, ybuf, abuf)     # h1 -> abuf
            nc.sync.dma_start(
                out=out_h[:].rearrange("(t p) d -> p t d", p=P),
                in_=abuf[:].rearrange("p (t d) -> p t d", d=H))
            # ---------------- pooling ----------------
            pls = spool.tile([P, NW * H], f32, tag="pls")
            nc.vector.memset(pls[:], 0.0)
            for t in range(NT):
                hb = spool.tile([P, H], bf16, tag="hbf")
                nc.vector.tensor_copy(out=hb[:], in_=abuf[:, t * H:(t + 1) * H])
                pmk = mpool.tile([P, NW * P], bf16, tag="pmk")
                nc.vector.tensor_tensor(
                    out=pmk[:].rearrange("p (w j) -> p w j", j=P),
                    in0=boffs[:, t * NW:(t + 1) * NW].to_broadcast([P, NW, P]),
                    in1=iota[:].rearrange("p (c j) -> p c j", c=1)
                        .to_broadcast([P, NW, P]),
                    op=nc_alu("is_equal"))
                for w in range(NW):
                    pw = ppool.tile([P, H], f32, tag="pool")
                    nc.tensor.matmul(pw[:], lhsT=pmk[:, w * P:(w + 1) * P],
                                     rhs=hb[:], start=True, stop=True)
                    nc.vector.tensor_add(out=pls[:, w * H:(w + 1) * H],
                                         in0=pls[:, w * H:(w + 1) * H],
                                         in1=pw[:])
            nc.sync.dma_start(
                out=ccp_in[:].rearrange("(w p) d -> p w d", p=P),
                in_=pls[:].rearrange("p (w d) -> p w d", d=H))
            nc.gpsimd.collective_compute(
                "AllReduce", nc_alu("add"), replica_groups=RG,
                ins=[ccp_in[:]], outs=[ccp_out[:]])
            xgt = spool.tile([P, NW * H], f32, tag="xgt")
            nc.sync.dma_start(
                out=xgt[:].rearrange("p (w d) -> p w d", d=H),
                in_=ccp_out[:].rearrange("(w p) d -> p w d", p=P))
            nc.sync.dma_start(
                out=out_xg[:].rearrange("(w p) d -> p w d", p=P),
                in_=xgt[:].rearrange("p (w d) -> p w d", d=H))

    nc.compile()
    return nc


def make_in_maps(prep, x, weights):
    """weights: dict name->array (W1_0, b1_0, g_0, be_0, W2_0, b2_0, *_1)."""
    import ml_dtypes

    def bf(a):
        return np.asarray(a).astype(ml_dtypes.bfloat16)

    x = np.asarray(x, np.float32)
    iota = bf(np.tile(np.arange(P, dtype=np.float32), (P, 1)))
    ident = np.eye(P, dtype=np.float32)
    ones = np.ones((P, 1), np.float32)
    pmask = (np.arange(P) < (NSH - (NT - 1) * P)).astype(np.float32)[:, None]

    common = {"iota": iota, "ident": ident, "ones": ones, "pmask": pmask}
    for l in range(2):
        common[f"W1_{l}"] = np.asarray(weights[f"W1_{l}"], np.float32)
        common[f"W2_{l}"] = np.asarray(weights[f"W2_{l}"], np.float32)
        common[f"b1_{l}"] = np.tile(np.asarray(weights[f"b1_{l}"], np.float32),
                                    (P, 1))
        common[f"b2_{l}"] = np.tile(np.asarray(weights[f"b2_{l}"], np.float32),
                                    (P, 1))
        common[f"g_{l}"] = np.asarray(weights[f"g_{l}"], np.float32)[:, None]
        common[f"be_{l}"] = np.asarray(weights[f"be_{l}"], np.float32)[:, None]

    in_maps = []
    for c in range(NCORES):
        cd = prep["cores"][c]
        xs = np.zeros((NS, D_IN), np.float32)
        xs[:NSH] = x[c * NSH:(c + 1) * NSH]
        m = dict(common)
        m["xT"] = np.ascontiguousarray(xs.T)
        m["gidx"] = cd["gidx"]
        m["dloc"] = bf(cd["dloc"].astype(np.float32))
        m["boff"] = bf(np.clip(cd["boff"], -30000, 30000).astype(np.float32))
        in_maps.append(m)
    return in_maps


_CACHE = {}


def kernel(x, edge_index, batch,
           W1_0, b1_0, g_0, be_0, W2_0, b2_0,
           W1_1, b1_1, g_1, be_1, W2_1, b2_1):
    from concourse.bass_utils import run_bass_kernel_spmd

    prep = preprocess(edge_index, batch)
    key = (prep["idx_cols"], prep["chunk_cols"],
           tuple(tuple(r) for s in prep["runs"] for r in s))
    if key not in _CACHE:
        _CACHE[key] = build_nc(prep)
    nc = _CACHE[key]

    weights = dict(W1_0=W1_0, b1_0=b1_0, g_0=g_0, be_0=be_0, W2_0=W2_0,
                   b2_0=b2_0, W1_1=W1_1, b1_1=b1_1, g_1=g_1, be_1=be_1,
                   W2_1=W2_1, b2_1=b2_1)
    in_maps = make_in_maps(prep, x, weights)
    res = run_bass_kernel_spmd(nc, in_maps, core_ids=list(range(NCORES)),
                               trace=False)
    h = np.concatenate([res.results[c]["out_h"][:NSH] for c in range(NCORES)],
                       axis=0).astype(np.float32)
    xg = np.asarray(res.results[0]["out_xg"], np.float32)
    return (xg, h)


# revision 12
# speedup vs baseline: 3.6722x; 3.6722x over previous
"""GIN 2-layer encoder (gnn_message_passing) on 8 TRN2 NeuronCores, Bass/Tile SPMD.

Strategy (sharding_hint: partition nodes + incident edges, replicate params,
all-reduce BN stats, exchange activations):
- Nodes sharded 12500/core, padded to 12544 = 98*128. Edges assigned to the
  core owning dst.
- GIN linearity: mlp(h + agg(h)) with first op Linear => y = h@W1 per node,
  then z = y + segsum(y[src]) + b1. So per layer: local y matmul -> AllGather
  y table -> gather y[src] rows (dma_gather, 256B rows, 4 int16 subtable
  passes) -> segment-sum into per-dst-tile PSUM via is_equal masks (bf16) +
  PE matmuls (no scatter; dst grouped by 128-node tile on host).
- BN (training stats over all N): per-feature sum/sumsq via ones-matmul,
  AllReduce, fold scale/shift applied in transposed space before the W2 matmul.
- Global add pool: mask matmuls vs batch ids into 8 global graph windows
  (psum [128, 8*64]) + AllReduce.
SPMD: one program for all cores; per-(subtable, dst-tile) chunk counts are
padded to the max across cores so the structure is identical; per-core data
(indices, mask columns) differs only in tensor contents.
Host preprocessing is index-only (grouping/sorting/layout of edge indices).
"""
import sys
sys.path.insert(0, "/opt/trn_rl_repo")

import numpy as np

# ---------------- dimensions (hardcoded per contract) ----------------
N = 100000
E = 3200000
G = 1024
D_IN = 128
H = 64
BN_EPS = 1e-5
NCORES = 8

P = 128
KG = 4096              # idxs per dma_gather call
SUBC = KG // P         # sub-chunks per gather (32)
NSUB = 4               # gather subtables (int16 idx limit)


def _derived():
    global NSH, NS, NT, NFULL, ST, NW
    NSH = N // NCORES                      # real nodes per core
    NS = ((NSH + P - 1) // P) * P          # padded nodes per core
    NT = NS // P                           # node tiles per core
    NFULL = NS * NCORES                    # padded total
    assert NFULL % NSUB == 0
    ST = NFULL // NSUB                     # subtable rows
    assert ST <= 32768, ST
    NW = (G + P - 1) // P                  # pooling windows (global)


_derived()


def _wrap_idxs(idx):
    """[K] -> [128, K/16] int16: idx j at partition j%16, col j//16, x8 repl."""
    k = idx.shape[0]
    w = idx.astype(np.int16).reshape(k // 16, 16).T
    return np.tile(w, (8, 1))


def preprocess(edge_index, batch):
    """Index-only host prep -> (structure, per-core arrays)."""
    src = np.asarray(edge_index[0], np.int64)
    dst = np.asarray(edge_index[1], np.int64)
    batch = np.asarray(batch, np.int64)
    core = dst // NSH
    dl = dst - core * NSH
    srcp = (src // NSH) * NS + (src % NSH)
    s_tab = srcp // ST
    s_loc = srcp - s_tab * ST
    t_dst = dl // P
    w_dst = dl - t_dst * P

    # per-core, per-(s,t) edge lists
    counts = np.zeros((NCORES, NSUB, NT), np.int64)
    per = {}
    for c in range(NCORES):
        m = core == c
        key = (s_tab[m] * NT + t_dst[m]).astype(np.int64)
        order = np.argsort(key, kind="stable")
        sl_, td_, wd_, key_ = s_loc[m][order], t_dst[m][order], w_dst[m][order], key[order]
        ug, starts, cnts = np.unique(key_, return_index=True, return_counts=True)
        counts[c][ug // NT, ug % NT] = cnts
        per[c] = (sl_, wd_, {int(g): (int(a), int(n))
                             for g, a, n in zip(ug, starts, cnts)})

    # uniform chunk counts: max over cores, >= 1
    nch = np.maximum(1, (counts.max(axis=0) + P - 1) // P)  # [NSUB, NT]
    runs = [[(t, int(nch[s, t])) for t in range(NT)] for s in range(NSUB)]
    ng = []
    for s in range(NSUB):
        tot = int(nch[s].sum()) * P
        ng.append((tot + KG - 1) // KG)

    cores = []
    for c in range(NCORES):
        sl_, wd_, groups = per[c]
        gidx_parts, dl_parts = [], []
        for s in range(NSUB):
            ilen = 0
            for t in range(NT):
                g = s * NT + t
                a, n = groups.get(g, (0, 0))
                want = int(nch[s, t]) * P
                ii = np.zeros(want, np.int64)
                dd = np.full(want, 999, np.int64)
                ii[:n] = sl_[a:a + n]
                dd[:n] = wd_[a:a + n]
                gidx_parts.append(ii)
                dl_parts.append(dd)
                ilen += want
            pad = ng[s] * KG - ilen
            if pad:
                gidx_parts.append(np.zeros(pad, np.int64))
                dl_parts.append(np.full(pad, 999, np.int64))
        gidx = np.concatenate(gidx_parts)
        dloc = np.concatenate(dl_parts)
        nchk = dloc.shape[0] // P
        dloc_t = dloc.reshape(nchk, P).T  # [128, nchunks]

        bown = batch[c * NSH:(c + 1) * NSH]
        bpad = np.concatenate([bown, np.full(NS - NSH, 10 ** 6)])
        boff = np.zeros((P, NT * NW), np.int64)
        for t in range(NT):
            seg = bpad[t * P:(t + 1) * P]
            for w in range(NW):
                boff[:, t * NW + w] = seg - w * P
        cores.append(dict(gidx=_wrap_idxs(gidx), dloc=dloc_t, boff=boff))
    return dict(runs=runs, ng=ng, cores=cores,
                idx_cols=cores[0]["gidx"].shape[1],
                chunk_cols=cores[0]["dloc"].shape[1])


def build_nc(struct, stub=False):
    from concourse import mybir
    from concourse import bacc
    import concourse.tile as tile
    import contextlib

    f32 = mybir.dt.float32
    bf16 = mybir.dt.bfloat16
    i16 = mybir.dt.int16
    AF = mybir.ActivationFunctionType

    runs, ng = struct["runs"], struct["ng"]
    idx_cols, chunk_cols = struct["idx_cols"], struct["chunk_cols"]
    max_ng = max(ng) if ng else 1

    nc = bacc.Bacc("TRN2", target_bir_lowering=False, debug=False,
                   num_devices=NCORES)

    xT = nc.dram_tensor("xT", [D_IN, NS], f32, kind="ExternalInput")
    Wd = {}
    for l in range(2):
        kin = D_IN if l == 0 else H
        for nm, shp in (("W1", [kin, H]), ("W2", [H, H]), ("b1", [P, H]),
                        ("b2", [P, H]), ("g", [H, 1]), ("be", [H, 1])):
            Wd[(nm, l)] = nc.dram_tensor(f"{nm}_{l}", shp, f32,
                                         kind="ExternalInput")
    iota_d = nc.dram_tensor("iota", [P, P], bf16, kind="ExternalInput")
    ident_d = nc.dram_tensor("ident", [P, P], f32, kind="ExternalInput")
    ones_d = nc.dram_tensor("ones", [P, 1], f32, kind="ExternalInput")
    pmask_d = nc.dram_tensor("pmask", [P, 1], f32, kind="ExternalInput")
    gidx_d = nc.dram_tensor("gidx", [P, idx_cols], i16, kind="ExternalInput")
    dloc_d = nc.dram_tensor("dloc", [P, chunk_cols], bf16, kind="ExternalInput")
    boff_d = nc.dram_tensor("boff", [P, NT * NW], bf16, kind="ExternalInput")

    out_h = nc.dram_tensor("out_h", [NS, H], f32, kind="ExternalOutput")
    out_xg = nc.dram_tensor("out_xg", [G, H], f32, kind="ExternalOutput")

    ccy_in = [nc.dram_tensor(f"ccy_in{l}", [NS, H], f32) for l in range(2)]
    ccy_out = [nc.dram_tensor(f"ccy_out{l}", [NFULL, H], f32,
                              addr_space="Shared") for l in range(2)]
    ccs_in = [nc.dram_tensor(f"ccs_in{l}", [H, 2], f32) for l in range(2)]
    ccs_out = [nc.dram_tensor(f"ccs_out{l}", [H, 2], f32,
                              addr_space="Shared") for l in range(2)]
    ccp_in = nc.dram_tensor("ccp_in", [G, H], f32)
    ccp_out = nc.dram_tensor("ccp_out", [G, H], f32, addr_space="Shared")

    RG = [list(range(NCORES))]

    if stub:
        with tile.TileContext(nc) as tc:
            with tc.tile_pool(name="z", bufs=1) as zp:
                zt = zp.tile([P, NT * H], mybir.dt.float32)
                nc.vector.memset(zt[:], 0.0)
                nc.sync.dma_start(
                    out=out_h[:].rearrange("(t p) d -> p t d", p=P),
                    in_=zt[:].rearrange("p (t d) -> p t d", d=H))
                zg = zp.tile([P, NW * H], mybir.dt.float32)
                nc.vector.memset(zg[:], 0.0)
                nc.sync.dma_start(
                    out=out_xg[:].rearrange("(w p) d -> p w d", p=P),
                    in_=zg[:].rearrange("p (w d) -> p w d", d=H))
        nc.compile()
        return nc

    with tile.TileContext(nc) as tc:
        with contextlib.ExitStack() as ctx:
            cpool = ctx.enter_context(tc.tile_pool(name="const", bufs=1))
            big = ctx.enter_context(tc.tile_pool(name="big", bufs=1))
            xpool = ctx.enter_context(tc.tile_pool(name="xs", bufs=2))
            ipool = ctx.enter_context(tc.tile_pool(name="idx", bufs=2))
            vpool = ctx.enter_context(tc.tile_pool(name="vals", bufs=4))
            bpool = ctx.enter_context(tc.tile_pool(name="valsbf", bufs=3))
            mpool = ctx.enter_context(tc.tile_pool(name="masks", bufs=3))
            spool = ctx.enter_context(tc.tile_pool(name="small", bufs=3))
            pacc = ctx.enter_context(tc.tile_pool(name="pacc", bufs=3,
                                                  space="PSUM"))
            pmisc = ctx.enter_context(tc.tile_pool(name="pmisc", bufs=1,
                                                   space="PSUM"))
            pstat = ctx.enter_context(tc.tile_pool(name="pstat", bufs=1,
                                                   space="PSUM"))
            ppool = ctx.enter_context(tc.tile_pool(name="ppool", bufs=2,
                                                   space="PSUM"))

            iota = cpool.tile([P, P], bf16)
            nc.sync.dma_start(out=iota[:], in_=iota_d[:])
            ident = cpool.tile([P, P], f32)
            nc.sync.dma_start(out=ident[:], in_=ident_d[:])
            onesc = cpool.tile([P, 1], f32)
            nc.sync.dma_start(out=onesc[:], in_=ones_d[:])
            pmask = cpool.tile([P, 1], f32)
            nc.sync.dma_start(out=pmask[:], in_=pmask_d[:])
            dloc = cpool.tile([P, chunk_cols], bf16)
            nc.sync.dma_start(out=dloc[:], in_=dloc_d[:])
            boffs = cpool.tile([P, NT * NW], bf16)
            nc.sync.dma_start(out=boffs[:], in_=boff_d[:])
            wt = {}
            for l in range(2):
                kin = D_IN if l == 0 else H
                for nm, shp in (("W1", [kin, H]), ("W2", [H, H]),
                                ("b1", [P, H]), ("b2", [P, H]),
                                ("g", [H, 1]), ("be", [H, 1])):
                    t_ = cpool.tile(shp, f32, tag=f"{nm}_{l}")
                    nc.sync.dma_start(out=t_[:], in_=Wd[(nm, l)][:])
                    wt[(nm, l)] = t_

            ybuf = big.tile([P, NT * H], f32, tag="ybuf")
            abuf = big.tile([P, NT * H], f32, tag="abuf")

            XG = 8  # x tiles loaded per DMA

            def y_from_x():
                for t0 in range(0, NT, XG):
                    n = min(XG, NT - t0)
                    xt = xpool.tile([P, XG * P], f32, tag="xT")
                    nc.sync.dma_start(out=xt[:, :n * P],
                                      in_=xT[:, t0 * P:(t0 + n) * P])
                    for k in range(n):
                        t = t0 + k
                        ps = pmisc.tile([P, H], f32, tag="mp")
                        nc.tensor.matmul(ps[:], lhsT=xt[:, k * P:(k + 1) * P],
                                         rhs=wt[("W1", 0)][:],
                                         start=True, stop=True)
                        nc.vector.tensor_copy(out=ybuf[:, t * H:(t + 1) * H],
                                              in_=ps[:])

            def y_from_h(l, hsrc, ydst):
                for t in range(NT):
                    ps = pmisc.tile([H, P], f32, tag="mp")
                    nc.tensor.transpose(out=ps[:],
                                        in_=hsrc[:, t * H:(t + 1) * H],
                                        identity=ident[:])
                    hT = spool.tile([H, P], f32, tag="hT")
                    nc.vector.tensor_copy(out=hT[:], in_=ps[:])
                    ps2 = pmisc.tile([P, H], f32, tag="mp")
                    nc.tensor.matmul(ps2[:], lhsT=hT[:], rhs=wt[("W1", l)][:],
                                     start=True, stop=True)
                    nc.vector.tensor_copy(out=ydst[:, t * H:(t + 1) * H],
                                          in_=ps2[:])

            def allgather_y(l, ysrc):
                nc.sync.dma_start(
                    out=ccy_in[l][:].rearrange("(t p) d -> p t d", p=P),
                    in_=ysrc[:].rearrange("p (t d) -> p t d", d=H))
                nc.gpsimd.collective_compute(
                    "AllGather", nc_alu("bypass"), replica_groups=RG,
                    ins=[ccy_in[l][:]], outs=[ccy_out[l][:]])

            def nc_alu(name):
                return getattr(mybir.AluOpType, name)

            def edge_phase(l, aggdst):
                nc.vector.memset(aggdst[:], 0.0)
                yfull = ccy_out[l]
                icol = 0   # idx col offset in gidx_d (16 idx / col)
                cbase = 0  # chunk col offset in dloc
                for s in range(NSUB):
                    if ng[s] == 0:
                        continue
                    ncols = ng[s] * KG // 16
                    idxt = ipool.tile([P, max_ng * KG // 16], i16, tag="gidx")
                    nc.sync.dma_start(out=idxt[:, :ncols],
                                      in_=gidx_d[:, icol:icol + ncols])
                    chunk_meta = []
                    for t, nchk in runs[s]:
                        for k in range(nchk):
                            chunk_meta.append((t, k == 0, k == nchk - 1))
                    ps_cur = None
                    for gci in range(ng[s]):
                        vals = vpool.tile([P, SUBC * H], f32, tag="vals")
                        nc.gpsimd.dma_gather(
                            vals[:].rearrange("p (c d) -> p c d", d=H),
                            yfull[s * ST:(s + 1) * ST, :],
                            idxt[:, gci * KG // 16:(gci + 1) * KG // 16],
                            KG, KG, H, single_packet=False)
                        vb = bpool.tile([P, SUBC * H], bf16, tag="valsbf")
                        nc.vector.tensor_copy(out=vb[:], in_=vals[:])
                        mk = mpool.tile([P, SUBC * P], bf16, tag="mask")
                        cb = cbase + gci * SUBC
                        nc.vector.tensor_tensor(
                            out=mk[:].rearrange("p (c j) -> p c j", j=P),
                            in0=dloc[:, cb:cb + SUBC].to_broadcast([P, SUBC, P]),
                            in1=iota[:].rearrange("p (c j) -> p c j", c=1)
                                .to_broadcast([P, SUBC, P]),
                            op=nc_alu("is_equal"))
                        for loc in range(SUBC):
                            ci = gci * SUBC + loc
                            if ci >= len(chunk_meta):
                                break
                            t, first, last = chunk_meta[ci]
                            if first:
                                ps_cur = pacc.tile([P, H], f32, tag="acc")
                            nc.tensor.matmul(
                                ps_cur[:], lhsT=mk[:, loc * P:(loc + 1) * P],
                                rhs=vb[:, loc * H:(loc + 1) * H],
                                start=first, stop=last)
                            if last:
                                nc.vector.tensor_add(
                                    out=aggdst[:, t * H:(t + 1) * H],
                                    in0=aggdst[:, t * H:(t + 1) * H],
                                    in1=ps_cur[:])
                    icol += ncols
                    cbase += ng[s] * SUBC

            def dense_phase(l, ysrc, aggbuf, hdst):
                sums0 = pstat.tile([H, 1], f32, tag="stats0")
                sums1 = pstat.tile([H, 1], f32, tag="stats1")
                for t in range(NT):
                    a = aggbuf[:, t * H:(t + 1) * H]
                    nc.vector.tensor_add(out=a, in0=a,
                                         in1=ysrc[:, t * H:(t + 1) * H])
                    nc.vector.tensor_add(out=a, in0=a, in1=wt[("b1", l)][:])
                    nc.scalar.activation(out=a, in_=a, func=AF.Relu)
                    if t == NT - 1:
                        nc.vector.tensor_mul(out=a, in0=a,
                                             in1=pmask[:].to_broadcast([P, H]))
                    sq = spool.tile([P, H], f32, tag="sq")
                    nc.vector.tensor_mul(out=sq[:], in0=a, in1=a)
                    nc.tensor.matmul(sums0[:], lhsT=a, rhs=onesc[:],
                                     start=(t == 0), stop=(t == NT - 1))
                    nc.tensor.matmul(sums1[:], lhsT=sq[:], rhs=onesc[:],
                                     start=(t == 0), stop=(t == NT - 1))
                st_sb = spool.tile([H, 2], f32, tag="stsb")
                nc.vector.tensor_copy(out=st_sb[:, 0:1], in_=sums0[:])
                nc.vector.tensor_copy(out=st_sb[:, 1:2], in_=sums1[:])
                nc.sync.dma_start(out=ccs_in[l][:], in_=st_sb[:])
                nc.gpsimd.collective_compute(
                    "AllReduce", nc_alu("add"), replica_groups=RG,
                    ins=[ccs_in[l][:]], outs=[ccs_out[l][:]])
                st2 = spool.tile([H, 2], f32, tag="st2")
                nc.sync.dma_start(out=st2[:], in_=ccs_out[l][:])
                mean = spool.tile([H, 1], f32, tag="mean")
                nc.vector.tensor_scalar_mul(out=mean[:], in0=st2[:, 0:1],
                                            scalar1=1.0 / N)
                ex2 = spool.tile([H, 1], f32, tag="ex2")
                nc.vector.tensor_scalar_mul(out=ex2[:], in0=st2[:, 1:2],
                                            scalar1=1.0 / N)
                var = spool.tile([H, 1], f32, tag="var")
                nc.vector.tensor_mul(out=var[:], in0=mean[:], in1=mean[:])
                nc.vector.tensor_sub(out=var[:], in0=ex2[:], in1=var[:])
                nc.vector.tensor_scalar_add(out=var[:], in0=var[:],
                                            scalar1=BN_EPS)
                rstd = spool.tile([H, 1], f32, tag="rstd")
                nc.scalar.activation(out=rstd[:], in_=var[:], func=AF.Sqrt)
                nc.vector.reciprocal(out=rstd[:], in_=rstd[:])
                scale = spool.tile([H, 1], f32, tag="scale")
                nc.vector.tensor_mul(out=scale[:], in0=wt[("g", l)][:],
                                     in1=rstd[:])
                shift = spool.tile([H, 1], f32, tag="shift")
                nc.vector.tensor_mul(out=shift[:], in0=mean[:], in1=scale[:])
                nc.vector.tensor_sub(out=shift[:], in0=wt[("be", l)][:],
                                     in1=shift[:])
                for t in range(NT):
                    ps = pmisc.tile([H, P], f32, tag="mp")
                    nc.tensor.transpose(out=ps[:],
                                        in_=aggbuf[:, t * H:(t + 1) * H],
                                        identity=ident[:])
                    aT = spool.tile([H, P], f32, tag="aT")
                    nc.vector.tensor_mul(out=aT[:], in0=ps[:],
                                         in1=scale[:].to_broadcast([H, P]))
                    nc.vector.tensor_add(out=aT[:], in0=aT[:],
                                         in1=shift[:].to_broadcast([H, P]))
                    ps2 = pmisc.tile([P, H], f32, tag="mp")
                    nc.tensor.matmul(ps2[:], lhsT=aT[:], rhs=wt[("W2", l)][:],
                                     start=True, stop=True)
                    hsl = hdst[:, t * H:(t + 1) * H]
                    nc.vector.tensor_add(out=hsl, in0=ps2[:],
                                         in1=wt[("b2", l)][:])
                    nc.scalar.activation(out=hsl, in_=hsl, func=AF.Relu)

            # ---------------- layers ----------------
            y_from_x()
            allgather_y(0, ybuf)
            edge_phase(0, abuf)
            dense_phase(0, ybuf, abuf, ybuf)     # h0 -> ybuf
            y_from_h(1, ybuf, abuf)              # y2 -> abuf
            allgather_y(1, abuf)
            edge_phase(1, ybuf)                  # agg2 -> ybuf (h0 dead)
            dense_phase(1, abuf, ybuf, abuf)     # h1 -> abuf
            nc.sync.dma_start(
                out=out_h[:].rearrange("(t p) d -> p t d", p=P),
                in_=abuf[:].rearrange("p (t d) -> p t d", d=H))
            # ---------------- pooling ----------------
            pls = spool.tile([P, NW * H], f32, tag="pls")
            nc.vector.memset(pls[:], 0.0)
            for t in range(NT):
                hb = spool.tile([P, H], bf16, tag="hbf")
                nc.vector.tensor_copy(out=hb[:], in_=abuf[:, t * H:(t + 1) * H])
                pmk = mpool.tile([P, NW * P], bf16, tag="pmk")
                nc.vector.tensor_tensor(
                    out=pmk[:].rearrange("p (w j) -> p w j", j=P),
                    in0=boffs[:, t * NW:(t + 1) * NW].to_broadcast([P, NW, P]),
                    in1=iota[:].rearrange("p (c j) -> p c j", c=1)
                        .to_broadcast([P, NW, P]),
                    op=nc_alu("is_equal"))
                for w in range(NW):
                    pw = ppool.tile([P, H], f32, tag="pool")
                    nc.tensor.matmul(pw[:], lhsT=pmk[:, w * P:(w + 1) * P],
                                     rhs=hb[:], start=True, stop=True)
                    nc.vector.tensor_add(out=pls[:, w * H:(w + 1) * H],
                                         in0=pls[:, w * H:(w + 1) * H],
                                         in1=pw[:])
            nc.sync.dma_start(
                out=ccp_in[:].rearrange("(w p) d -> p w d", p=P),
                in_=pls[:].rearrange("p (w d) -> p w d", d=H))
            nc.gpsimd.collective_compute(
                "AllReduce", nc_alu("add"), replica_groups=RG,
                ins=[ccp_in[:]], outs=[ccp_out[:]])
            xgt = spool.tile([P, NW * H], f32, tag="xgt")
            nc.sync.dma_start(
                out=xgt[:].rearrange("p (w d) -> p w d", d=H),
                in_=ccp_out[:].rearrange("(w p) d -> p w d", p=P))
            nc.sync.dma_start(
                out=out_xg[:].rearrange("(w p) d -> p w d", p=P),
                in_=xgt[:].rearrange("p (w d) -> p w d", d=H))

    nc.compile()
    return nc


def make_in_maps(prep, x, weights):
    """weights: dict name->array (W1_0, b1_0, g_0, be_0, W2_0, b2_0, *_1)."""
    import ml_dtypes

    def bf(a):
        return np.asarray(a).astype(ml_dtypes.bfloat16)

    x = np.asarray(x, np.float32)
    iota = bf(np.tile(np.arange(P, dtype=np.float32), (P, 1)))
    ident = np.eye(P, dtype=np.float32)
    ones = np.ones((P, 1), np.float32)
    pmask = (np.arange(P) < (NSH - (NT - 1) * P)).astype(np.float32)[:, None]

    common = {"iota": iota, "ident": ident, "ones": ones, "pmask": pmask}
    for l in range(2):
        common[f"W1_{l}"] = np.asarray(weights[f"W1_{l}"], np.float32)
        common[f"W2_{l}"] = np.asarray(weights[f"W2_{l}"], np.float32)
        common[f"b1_{l}"] = np.tile(np.asarray(weights[f"b1_{l}"], np.float32),
                                    (P, 1))
        common[f"b2_{l}"] = np.tile(np.asarray(weights[f"b2_{l}"], np.float32),
                                    (P, 1))
        common[f"g_{l}"] = np.asarray(weights[f"g_{l}"], np.float32)[:, None]
        common[f"be_{l}"] = np.asarray(weights[f"be_{l}"], np.float32)[:, None]

    in_maps = []
    for c in range(NCORES):
        cd = prep["cores"][c]
        xs = np.zeros((NS, D_IN), np.float32)
        xs[:NSH] = x[c * NSH:(c + 1) * NSH]
        m = dict(common)
        m["xT"] = np.ascontiguousarray(xs.T)
        m["gidx"] = cd["gidx"]
        m["dloc"] = bf(cd["dloc"].astype(np.float32))
        m["boff"] = bf(np.clip(cd["boff"], -30000, 30000).astype(np.float32))
        in_maps.append(m)
    return in_maps


_CACHE = {}


def kernel(x, edge_index, batch,
           W1_0, b1_0, g_0, be_0, W2_0, b2_0,
           W1_1, b1_1, g_1, be_1, W2_1, b2_1):
    from concourse.bass_utils import run_bass_kernel_spmd

    prep = preprocess(edge_index, batch)
    key = (prep["idx_cols"], prep["chunk_cols"],
           tuple(tuple(r) for s in prep["runs"] for r in s))
    if key not in _CACHE:
        _CACHE[key] = build_nc(prep)
    nc = _CACHE[key]

    weights = dict(W1_0=W1_0, b1_0=b1_0, g_0=g_0, be_0=be_0, W2_0=W2_0,
                   b2_0=b2_0, W1_1=W1_1, b1_1=b1_1, g_1=g_1, be_1=be_1,
                   W2_1=W2_1, b2_1=b2_1)
    in_maps = make_in_maps(prep, x, weights)
    import time as _time
    last_err = None
    for attempt in range(3):
        try:
            res = run_bass_kernel_spmd(nc, in_maps,
                                       core_ids=list(range(NCORES)),
                                       trace=False)
            break
        except Exception as e:  # transient device-unrecoverable after crashes
            last_err = e
            _time.sleep(60 * (attempt + 1))
    else:
        raise last_err
    h = np.concatenate([res.results[c]["out_h"][:NSH] for c in range(NCORES)],
                       axis=0).astype(np.float32)
    xg = np.asarray(res.results[0]["out_xg"], np.float32)
    return (xg, h)
